# revision 15
# baseline (speedup 1.0000x reference)
"""KoLeo loss kernel for Trainium2 (8 NeuronCores, Bass/Tile).

fp8 DoubleRow + symmetric-Gram edition.

reference semantics:
    x = student_output / max(||row||_2, 1e-8)        # [B, D] row-normalize
    dots = x @ x.T ; dots[i,i] = -1
    nn = argmax(dots, axis=1)
    d_i = || x_i - x_nn(i) + 1e-8 ||_2
    loss = mean(-log(d_i + 1e-8))

Strategy:
  * Host pre-normalizes rows in fp32, scales by S=128, quantizes to fp8
    e4m3 (TRN FP8_EXP4 max normal 240 > S) and ships the transposed
    layout [KT=8, 128, B].  End-to-end numpy-validated rel err 1.4e-4.
  * dots is symmetric: only the upper triangle of the 16x16 grid of
    [512 x 512] blocks is computed -- 136 blocks, 17 per core.  All
    cores run the IDENTICAL block template
        {(0,0), (8,8), (0,8)} + {(0,d), (8,8+d) : d=1..7}
    over a column-strip ROTATED copy of x (core c's strip s = global
    strip (s+c) mod 16).  The 8 rotations tile all 136 blocks exactly
    once (verified), so the NEFF is the same for every core and only
    the input data differs.
  * Each [512x512] block: 16 fp8 DoubleRow matmuls (2 k-tiles per MM,
    2x bf16 PE throughput) into 4 psum tiles [128,512].  ACT drains
    each psum tile to a bf16 SBUF copy; DVE max8 takes per-row tile
    maxima (row side); for off-diagonal blocks GPSIMD reduces the 4
    bf16 copies elementwise to macc[128,512] (column side), which is
    DMA'd to DRAM.
  * Host combine: for each global row, its NN dot is the max over the
    16 candidate values it receives (row-side tile maxima where the
    row's strip is the block's row side; partition-maxima of macc
    where it is the column side; for diagonal blocks the top-1 is the
    row's self-dot ~S^2 and the top-2 value is the candidate).  Then
    d^2 = 2 - 2 m~ / S^2, loss = mean(-0.5 log d^2).
"""

import numpy as np
import ml_dtypes

import concourse.bacc as bacc
import concourse.bass as bass
import concourse.mybir as mybir
import concourse.tile as tile
from concourse import bass_utils

B, D, P = 8192, 1024, 128
NCORES = 8
KT = D // P          # 8 contraction tiles
GS = 512             # strip size (block edge, also moving free dim)
NS = B // GS         # 16 strips
MT4 = GS // P        # 4 row chunks per block
SCALE = 128.0        # fp8 pre-scale; self-dot ~ S^2

# 17 blocks per core, ordered so the needed strips arrive incrementally
# and the LAST block is diagonal (cheapest drain tail: no mirror chain).
TEMPLATE = (
    [(0, 0)]
    + [(0, b) for b in range(1, 9)]
    + [(8, b) for b in range(9, 16)]
    + [(8, 8)]
)
NBLK = len(TEMPLATE)           # 17
OFF_SLOTS = [t for t, (a, b) in enumerate(TEMPLATE) if a != b]
NOFF = len(OFF_SLOTS)          # 15

F32 = mybir.dt.float32
BF16 = mybir.dt.bfloat16
FP8 = mybir.dt.float8e4
DR = mybir.MatmulPerfMode.DoubleRow


def emit_kernel(tc, x_ap, rowc_ap, macc_ap):
    nc = tc.nc
    with (
        tc.tile_pool(name="big", bufs=1) as big,
        tc.tile_pool(name="work", bufs=3) as work,
        tc.tile_pool(name="ps", bufs=2, space="PSUM") as pp,
    ):
        xT = big.tile([P, KT, B], FP8)
        # dedicated stationary-operand copy of strips 0 and 8 so LDWEIGHTS
        # reads never contend with the moving-operand reads of xT
        wT = big.tile([P, KT, 2, GS], FP8)
        rowc = big.tile([P, NBLK, MT4, 8], F32)
        warm = big.tile([P, GS], FP8)

        nc.vector.memset(warm[:], 1.0)

        # --- input DMA: k-granular, incremental strip order.  Strip 0 and
        # the weight strips first, split across both HWDGE queues, so block
        # (0,0) can start after ~4 small transfers; the rest stream on SP
        # well ahead of consumption.
        for k in range(KT):
            q = nc.sync if k % 2 == 0 else nc.scalar
            q.dma_start(out=xT[:, k, 0:GS], in_=x_ap[k, :, 0:GS])
            q2 = nc.scalar if k % 2 == 0 else nc.sync
            q2.dma_start(out=wT[:, k, 0], in_=x_ap[k, :, 0:GS])
        for k in range(KT):
            q = nc.sync if k % 2 == 0 else nc.scalar
            q.dma_start(out=xT[:, k, GS : 2 * GS], in_=x_ap[k, :, GS : 2 * GS])
            q2 = nc.scalar if k % 2 == 0 else nc.sync
            q2.dma_start(out=wT[:, k, 1], in_=x_ap[k, :, 8 * GS : 9 * GS])
        for ch in range(1, NS // 2):
            cb = slice(ch * 2 * GS, (ch + 1) * 2 * GS)
            for k in range(KT):
                nc.sync.dma_start(out=xT[:, k, cb], in_=x_ap[k, :, cb])

        # --- PE/HAM pre-warm on the memset tile during the first DMAs.
        wps = pp.tile([P, GS], F32, tag="ps_m0", name="wps")
        for _ in range(12):
            nc.tensor.matmul(wps[:], warm[:, :P], warm[:], start=True, stop=True)

        # --- 17 symmetric blocks ------------------------------------------
        noff = 0
        for t, (a, b) in enumerate(TEMPLATE):
            ai = 0 if a == 0 else 1
            pss = [
                pp.tile([P, GS], F32, tag=f"ps_m{mt}", name=f"ps_m{mt}")
                for mt in range(MT4)
            ]
            for kk in range(KT // 2):
                ks = slice(2 * kk, 2 * kk + 2)
                for mt in range(MT4):
                    nc.tensor.matmul(
                        pss[mt][:],
                        wT[:, ks, ai, mt * P : (mt + 1) * P],
                        xT[:, ks, b * GS : (b + 1) * GS],
                        start=(kk == 0),
                        stop=(kk == KT // 2 - 1),
                        perf_mode=DR,
                    )
            if a == b:
                # diagonal: row-side only, max8 straight from PSUM
                for mt in range(MT4):
                    nc.vector.max(out=rowc[:, t, mt], in_=pss[mt][:])
            else:
                cp = work.tile([P, MT4, GS], BF16, tag="cp", name="cp")
                for mt in range(MT4):
                    nc.scalar.copy(cp[:, mt], pss[mt][:])
                    nc.vector.max(out=rowc[:, t, mt], in_=cp[:, mt])
                t01 = work.tile([P, GS], BF16, tag="t01", name="t01")
                macc = work.tile([P, GS], BF16, tag="macc", name="macc")
                nc.vector.tensor_max(t01[:], cp[:, 0], cp[:, 1])
                nc.vector.tensor_max(macc[:], cp[:, 2], cp[:, 3])
                nc.vector.tensor_max(macc[:], macc[:], t01[:])
                nc.sync.dma_start(out=macc_ap[noff], in_=macc[:])
                noff += 1
            if t == 8:
                nc.scalar.dma_start(out=rowc_ap[:, 0:9], in_=rowc[:, 0:9])

        nc.scalar.dma_start(out=rowc_ap[:, 9:NBLK], in_=rowc[:, 9:NBLK])


def build_bass():
    nc = bacc.Bacc(
        "TRN2",
        target_bir_lowering=False,
        debug=False,
        enable_asserts=True,
        num_devices=NCORES,
    )
    x_t = nc.dram_tensor("xq", [KT, P, B], FP8, kind="ExternalInput").ap()
    rowc_t = nc.dram_tensor(
        "rowc", [P, NBLK, MT4, 8], F32, kind="ExternalOutput"
    ).ap()
    macc_t = nc.dram_tensor("macc", [NOFF, P, GS], BF16, kind="ExternalOutput").ap()
    with tile.TileContext(nc) as tc:
        emit_kernel(tc, x_t, rowc_t, macc_t)
    nc.compile()
    return nc


def make_in_maps(x: np.ndarray):
    norm = np.linalg.norm(x, axis=1, keepdims=True)
    xn = x / np.maximum(norm, 1e-8)
    q = (SCALE * xn).astype(ml_dtypes.float8_e4m3)
    # [KT, P, B]: element [k, p, r] = q[r, k*128 + p]  (transposed layout)
    xt = np.ascontiguousarray(q.reshape(B, KT, P).transpose(1, 2, 0))
    # core c sees the row axis rotated by c*GS: its strip s = global (s+c)%16
    return [
        {"xq": np.ascontiguousarray(np.roll(xt, -c * GS, axis=2))}
        for c in range(NCORES)
    ]


def reduce_outputs(results):
    cand = np.full((B, 16), -np.inf, np.float32)
    nsrc = np.zeros(B, np.int32)

    def put(rows, vals):
        cand[rows, nsrc[rows]] = vals
        nsrc[rows] += 1

    for c in range(NCORES):
        rowc = results[c]["rowc"].astype(np.float32)  # [P, NBLK, MT4, 8]
        macc = results[c]["macc"].astype(np.float32)  # [NOFF, P, GS]
        noff = 0
        for t, (a, b) in enumerate(TEMPLATE):
            ga, gb = (a + c) % NS, (b + c) % NS
            for mt in range(MT4):
                rows = np.arange(ga * GS + mt * P, ga * GS + (mt + 1) * P)
                if ga == gb:
                    # top-1 is the row's self-dot; top-2 is the candidate
                    put(rows, rowc[:, t, mt, 1])
                else:
                    put(rows, rowc[:, t, mt, 0])
            if ga != gb:
                rows = np.arange(gb * GS, (gb + 1) * GS)
                put(rows, macc[noff].max(axis=0))
                noff += 1

    assert (nsrc == 16).all()
    m2 = cand.max(axis=1).astype(np.float64)
    d2 = 2.0 - 2.0 * m2 / (SCALE * SCALE)
    loss = float(np.mean(-0.5 * np.log(d2)))
    return np.array(loss, dtype=np.float32)


_LAST_RESULTS = None  # BassKernelResults of the most recent run (for test.py)


def run(x: np.ndarray, trace: bool = False):
    global _LAST_RESULTS
    nc = build_bass()
    res = bass_utils.run_bass_kernel_spmd(
        nc,
        make_in_maps(x),
        core_ids=list(range(NCORES)),
        trace=trace,
        trace_cores=list(range(NCORES)) if trace else None,
    )
    _LAST_RESULTS = res
    return reduce_outputs(res.results)


def kernel(**inputs) -> np.ndarray:
    x = np.asarray(inputs["student_output"], dtype=np.float32)
    assert x.shape == (B, D), x.shape
    return run(x, trace=False)


if __name__ == "__main__":
    rng = np.random.default_rng(0)
    x = rng.standard_normal((B, D), dtype=np.float32)
    print(kernel(student_output=x))


# revision 18
# speedup vs baseline: 1.1640x; 1.1640x over previous
"""KoLeo loss kernel for Trainium2 (8 NeuronCores, Bass/Tile).

fp8 DoubleRow + symmetric-Gram edition.

reference semantics:
    x = student_output / max(||row||_2, 1e-8)        # [B, D] row-normalize
    dots = x @ x.T ; dots[i,i] = -1
    nn = argmax(dots, axis=1)
    d_i = || x_i - x_nn(i) + 1e-8 ||_2
    loss = mean(-log(d_i + 1e-8))

Strategy:
  * Host pre-normalizes rows in fp32, scales by S=128, quantizes to fp8
    e4m3 (TRN FP8_EXP4 max normal 240 > S) and ships the transposed
    layout [KT=8, 128, B].  End-to-end numpy-validated rel err 1.4e-4.
  * dots is symmetric: only the upper triangle of the 16x16 grid of
    [512 x 512] blocks is computed -- 136 blocks, 17 per core.  All
    cores run the IDENTICAL block template
        {(0,0), (8,8), (0,8)} + {(0,d), (8,8+d) : d=1..7}
    over a column-strip ROTATED copy of x (core c's strip s = global
    strip (s+c) mod 16).  The 8 rotations tile all 136 blocks exactly
    once (verified), so the NEFF is the same for every core and only
    the input data differs.
  * Each [512x512] block: 16 fp8 DoubleRow matmuls (2 k-tiles per MM,
    2x bf16 PE throughput) into 4 psum tiles [128,512].  ACT drains
    each psum tile to a bf16 SBUF copy; DVE max8 takes per-row tile
    maxima (row side); for off-diagonal blocks GPSIMD reduces the 4
    bf16 copies elementwise to macc[128,512] (column side), which is
    DMA'd to DRAM.
  * Host combine: for each global row, its NN dot is the max over the
    16 candidate values it receives (row-side tile maxima where the
    row's strip is the block's row side; partition-maxima of macc
    where it is the column side; for diagonal blocks the top-1 is the
    row's self-dot ~S^2 and the top-2 value is the candidate).  Then
    d^2 = 2 - 2 m~ / S^2, loss = mean(-0.5 log d^2).
"""

import numpy as np
import ml_dtypes

import concourse.bacc as bacc
import concourse.bass as bass
import concourse.mybir as mybir
import concourse.tile as tile
from concourse import bass_utils

B, D, P = 8192, 1024, 128
NCORES = 8
KT = D // P          # 8 contraction tiles
GS = 512             # strip size (block edge, also moving free dim)
NS = B // GS         # 16 strips
MT4 = GS // P        # 4 row chunks per block
SCALE = 128.0        # fp8 pre-scale; self-dot ~ S^2

# 17 blocks per core, ordered so the needed strips arrive incrementally
# and the LAST block is diagonal (cheapest drain tail: no mirror chain).
TEMPLATE = (
    [(0, 0)]
    + [(0, b) for b in range(1, 9)]
    + [(8, b) for b in range(9, 16)]
    + [(8, 8)]
)
NBLK = len(TEMPLATE)           # 17
OFF_SLOTS = [t for t, (a, b) in enumerate(TEMPLATE) if a != b]
NOFF = len(OFF_SLOTS)          # 15

F32 = mybir.dt.float32
BF16 = mybir.dt.bfloat16
FP8 = mybir.dt.float8e4
DR = mybir.MatmulPerfMode.DoubleRow


def emit_kernel(tc, x_ap, rowc_ap, macc_ap):
    nc = tc.nc
    with (
        tc.tile_pool(name="big", bufs=1) as big,
        tc.tile_pool(name="work", bufs=3) as work,
        tc.tile_pool(name="ps", bufs=2, space="PSUM") as pp,
    ):
        xT = big.tile([P, KT, B], FP8)
        # dedicated stationary-operand copy of strips 0 and 8 so LDWEIGHTS
        # reads never contend with the moving-operand reads of xT
        wT = big.tile([P, KT, 2, GS], FP8)
        rowc = big.tile([P, NBLK, MT4, 8], F32)
        # one dedicated macc slot per off-diagonal block: the outgoing DMAs
        # can lag arbitrarily without ever stalling the compute pipeline
        maccb = big.tile([P, NOFF, GS], BF16)
        warm = big.tile([P, GS], FP8)

        nc.vector.memset(warm[:], 1.0)

        # --- input DMA: one big multi-k DMA per 2-strip chunk (a single
        # InstDMACopy fans out across all 16 SDMA engines), all on the SP
        # queue so the ACT queue stays free for the psum-drain copies.
        # Order: weights strip 0, chunk(s0,s1), weights strip 8, chunks.
        nc.sync.dma_start(out=wT[:, :, 0], in_=x_ap[:, :, 0:GS])
        nc.sync.dma_start(out=xT[:, :, 0 : 2 * GS], in_=x_ap[:, :, 0 : 2 * GS])
        nc.sync.dma_start(out=wT[:, :, 1], in_=x_ap[:, :, 8 * GS : 9 * GS])
        for ch in range(1, NS // 2):
            cb = slice(ch * 2 * GS, (ch + 1) * 2 * GS)
            nc.sync.dma_start(out=xT[:, :, cb], in_=x_ap[:, :, cb])

        # --- PE/HAM pre-warm on the memset tile during the first DMAs.
        wps = pp.tile([P, GS], F32, tag="ps_m0", name="wps")
        for _ in range(12):
            nc.tensor.matmul(wps[:], warm[:, :P], warm[:], start=True, stop=True)

        # --- 17 symmetric blocks ------------------------------------------
        noff = 0
        for t, (a, b) in enumerate(TEMPLATE):
            ai = 0 if a == 0 else 1
            pss = [
                pp.tile([P, GS], F32, tag=f"ps_m{mt}", name=f"ps_m{mt}")
                for mt in range(MT4)
            ]
            for kk in range(KT // 2):
                ks = slice(2 * kk, 2 * kk + 2)
                for mt in range(MT4):
                    nc.tensor.matmul(
                        pss[mt][:],
                        wT[:, ks, ai, mt * P : (mt + 1) * P],
                        xT[:, ks, b * GS : (b + 1) * GS],
                        start=(kk == 0),
                        stop=(kk == KT // 2 - 1),
                        perf_mode=DR,
                    )
            if a == b:
                # diagonal: row-side only, max8 straight from PSUM
                for mt in range(MT4):
                    nc.vector.max(out=rowc[:, t, mt], in_=pss[mt][:])
            else:
                cp = work.tile([P, MT4, GS], BF16, tag="cp", name="cp")
                for mt in range(MT4):
                    nc.scalar.copy(cp[:, mt], pss[mt][:])
                    nc.vector.max(out=rowc[:, t, mt], in_=cp[:, mt])
                t01 = work.tile([P, GS], BF16, tag="t01", name="t01")
                nc.vector.tensor_max(t01[:], cp[:, 0], cp[:, 1])
                nc.vector.tensor_max(maccb[:, noff], cp[:, 2], cp[:, 3])
                nc.vector.tensor_max(maccb[:, noff], maccb[:, noff], t01[:])
                nc.sync.dma_start(out=macc_ap[noff], in_=maccb[:, noff])
                noff += 1
            if t == 8:
                nc.scalar.dma_start(out=rowc_ap[:, 0:9], in_=rowc[:, 0:9])

        nc.scalar.dma_start(out=rowc_ap[:, 9:NBLK], in_=rowc[:, 9:NBLK])


def build_bass():
    nc = bacc.Bacc(
        "TRN2",
        target_bir_lowering=False,
        debug=False,
        enable_asserts=True,
        num_devices=NCORES,
    )
    x_t = nc.dram_tensor("xq", [KT, P, B], FP8, kind="ExternalInput").ap()
    rowc_t = nc.dram_tensor(
        "rowc", [P, NBLK, MT4, 8], F32, kind="ExternalOutput"
    ).ap()
    macc_t = nc.dram_tensor("macc", [NOFF, P, GS], BF16, kind="ExternalOutput").ap()
    with tile.TileContext(nc) as tc:
        emit_kernel(tc, x_t, rowc_t, macc_t)
    nc.compile()
    return nc


def make_in_maps(x: np.ndarray):
    norm = np.linalg.norm(x, axis=1, keepdims=True)
    xn = x / np.maximum(norm, 1e-8)
    q = (SCALE * xn).astype(ml_dtypes.float8_e4m3)
    # [KT, P, B]: element [k, p, r] = q[r, k*128 + p]  (transposed layout)
    xt = np.ascontiguousarray(q.reshape(B, KT, P).transpose(1, 2, 0))
    # core c sees the row axis rotated by c*GS: its strip s = global (s+c)%16
    return [
        {"xq": np.ascontiguousarray(np.roll(xt, -c * GS, axis=2))}
        for c in range(NCORES)
    ]


def reduce_outputs(results):
    cand = np.full((B, 16), -np.inf, np.float32)
    nsrc = np.zeros(B, np.int32)

    def put(rows, vals):
        cand[rows, nsrc[rows]] = vals
        nsrc[rows] += 1

    for c in range(NCORES):
        rowc = results[c]["rowc"].astype(np.float32)  # [P, NBLK, MT4, 8]
        macc = results[c]["macc"].astype(np.float32)  # [NOFF, P, GS]
        noff = 0
        for t, (a, b) in enumerate(TEMPLATE):
            ga, gb = (a + c) % NS, (b + c) % NS
            for mt in range(MT4):
                rows = np.arange(ga * GS + mt * P, ga * GS + (mt + 1) * P)
                if ga == gb:
                    # top-1 is the row's self-dot; top-2 is the candidate
                    put(rows, rowc[:, t, mt, 1])
                else:
                    put(rows, rowc[:, t, mt, 0])
            if ga != gb:
                rows = np.arange(gb * GS, (gb + 1) * GS)
                put(rows, macc[noff].max(axis=0))
                noff += 1

    assert (nsrc == 16).all()
    m2 = cand.max(axis=1).astype(np.float64)
    d2 = 2.0 - 2.0 * m2 / (SCALE * SCALE)
    loss = float(np.mean(-0.5 * np.log(d2)))
    return np.array(loss, dtype=np.float32)


_LAST_RESULTS = None  # BassKernelResults of the most recent run (for test.py)


def run(x: np.ndarray, trace: bool = False):
    global _LAST_RESULTS
    nc = build_bass()
    res = bass_utils.run_bass_kernel_spmd(
        nc,
        make_in_maps(x),
        core_ids=list(range(NCORES)),
        trace=trace,
        trace_cores=list(range(NCORES)) if trace else None,
    )
    _LAST_RESULTS = res
    return reduce_outputs(res.results)


def kernel(**inputs) -> np.ndarray:
    x = np.asarray(inputs["student_output"], dtype=np.float32)
    assert x.shape == (B, D), x.shape
    return run(x, trace=False)


if __name__ == "__main__":
    rng = np.random.default_rng(0)
    x = rng.standard_normal((B, D), dtype=np.float32)
    print(kernel(student_output=x))


# revision 22
# speedup vs baseline: 1.2180x; 1.0464x over previous
"""KoLeo loss kernel for Trainium2 (8 NeuronCores, Bass/Tile).

fp8 DoubleRow + symmetric-Gram edition.

reference semantics:
    x = student_output / max(||row||_2, 1e-8)        # [B, D] row-normalize
    dots = x @ x.T ; dots[i,i] = -1
    nn = argmax(dots, axis=1)
    d_i = || x_i - x_nn(i) + 1e-8 ||_2
    loss = mean(-log(d_i + 1e-8))

Strategy:
  * Host pre-normalizes rows in fp32, scales by S=128, quantizes to fp8
    e4m3 (TRN FP8_EXP4 max normal 240 > S) and ships the transposed
    layout [KT=8, 128, B].  End-to-end numpy-validated rel err 1.4e-4.
  * dots is symmetric: only the upper triangle of the 16x16 grid of
    [512 x 512] blocks is computed -- 136 blocks, 17 per core.  All
    cores run the IDENTICAL block template
        {(0,0), (8,8), (0,8)} + {(0,d), (8,8+d) : d=1..7}
    over a column-strip ROTATED copy of x (core c's strip s = global
    strip (s+c) mod 16).  The 8 rotations tile all 136 blocks exactly
    once (verified), so the NEFF is the same for every core and only
    the input data differs.
  * Each [512x512] block: 16 fp8 DoubleRow matmuls (2 k-tiles per MM,
    2x bf16 PE throughput) into 4 psum tiles [128,512].  ACT drains
    each psum tile to a bf16 SBUF copy; DVE max8 takes per-row tile
    maxima (row side); for off-diagonal blocks GPSIMD reduces the 4
    bf16 copies elementwise to macc[128,512] (column side), which is
    DMA'd to DRAM.
  * Host combine: for each global row, its NN dot is the max over the
    16 candidate values it receives (row-side tile maxima where the
    row's strip is the block's row side; partition-maxima of macc
    where it is the column side; for diagonal blocks the top-1 is the
    row's self-dot ~S^2 and the top-2 value is the candidate).  Then
    d^2 = 2 - 2 m~ / S^2, loss = mean(-0.5 log d^2).
"""

import numpy as np
import ml_dtypes

import concourse.bacc as bacc
import concourse.bass as bass
import concourse.mybir as mybir
import concourse.tile as tile
from concourse import bass_utils

B, D, P = 8192, 1024, 128
NCORES = 8
KT = D // P          # 8 contraction tiles
GS = 512             # strip size (block edge, also moving free dim)
NS = B // GS         # 16 strips
MT4 = GS // P        # 4 row chunks per block
SCALE = 128.0        # fp8 pre-scale; self-dot ~ S^2

# 17 blocks per core, ordered so the needed strips arrive incrementally
# and the LAST block is diagonal (cheapest drain tail: no mirror chain).
TEMPLATE = (
    [(0, 0)]
    + [(0, b) for b in range(1, 9)]
    + [(8, b) for b in range(9, 16)]
    + [(8, 8)]
)
NBLK = len(TEMPLATE)           # 17
OFF_SLOTS = [t for t, (a, b) in enumerate(TEMPLATE) if a != b]
NOFF = len(OFF_SLOTS)          # 15

F32 = mybir.dt.float32
BF16 = mybir.dt.bfloat16
FP8 = mybir.dt.float8e4
DR = mybir.MatmulPerfMode.DoubleRow


def emit_kernel(tc, x_ap, rowc_ap, macc_ap):
    nc = tc.nc
    with (
        tc.tile_pool(name="big", bufs=1) as big,
        tc.tile_pool(name="work", bufs=4) as work,
        tc.tile_pool(name="ps", bufs=2, space="PSUM") as pp,
    ):
        xT = big.tile([P, KT, B], FP8)
        # dedicated stationary-operand copy of strips 0 and 8 so LDWEIGHTS
        # reads never contend with the moving-operand reads of xT
        wT = big.tile([P, KT, 2, GS], FP8)
        rowc = big.tile([P, NBLK, MT4, 8], F32)
        # one dedicated macc slot per off-diagonal block: the outgoing DMAs
        # can lag arbitrarily without ever stalling the compute pipeline
        maccb = big.tile([P, NOFF, GS], BF16)
        warm = big.tile([P, GS], FP8)

        nc.vector.memset(warm[:], 1.0)

        # --- input DMA: one big multi-k DMA per 2-strip chunk (a single
        # InstDMACopy fans out across all 16 SDMA engines), all on the SP
        # queue so the ACT queue stays free for the psum-drain copies.
        # Order: weights strip 0, chunk(s0,s1), weights strip 8, chunks.
        nc.sync.dma_start(out=xT[:, :, 0:GS], in_=x_ap[:, :, 0:GS])
        nc.sync.dma_start(out=wT[:, :, 0], in_=x_ap[:, :, 0:GS])
        nc.sync.dma_start(out=xT[:, :, GS : 2 * GS], in_=x_ap[:, :, GS : 2 * GS])
        nc.sync.dma_start(out=wT[:, :, 1], in_=x_ap[:, :, 8 * GS : 9 * GS])
        for ch in range(1, NS // 2):
            cb = slice(ch * 2 * GS, (ch + 1) * 2 * GS)
            nc.sync.dma_start(out=xT[:, :, cb], in_=x_ap[:, :, cb])

        # --- PE/HAM pre-warm on the memset tile during the first DMAs.
        wps = pp.tile([P, GS], F32, tag="ps_m0", name="wps")
        for _ in range(22):
            nc.tensor.matmul(wps[:], warm[:, :P], warm[:], start=True, stop=True)

        # --- 17 symmetric blocks ------------------------------------------
        noff = 0
        for t, (a, b) in enumerate(TEMPLATE):
            ai = 0 if a == 0 else 1
            pss = [
                pp.tile([P, GS], F32, tag=f"ps_m{mt}", name=f"ps_m{mt}")
                for mt in range(MT4)
            ]
            for kk in range(KT // 2):
                ks = slice(2 * kk, 2 * kk + 2)
                for mt in range(MT4):
                    nc.tensor.matmul(
                        pss[mt][:],
                        wT[:, ks, ai, mt * P : (mt + 1) * P],
                        xT[:, ks, b * GS : (b + 1) * GS],
                        start=(kk == 0),
                        stop=(kk == KT // 2 - 1),
                        perf_mode=DR,
                    )
            if a == b:
                # diagonal: row-side only, max8 straight from PSUM
                for mt in range(MT4):
                    nc.vector.max(out=rowc[:, t, mt], in_=pss[mt][:])
            else:
                cp = work.tile([P, MT4, GS], BF16, tag="cp", name="cp")
                for mt in range(MT4):
                    nc.scalar.copy(cp[:, mt], pss[mt][:])
                    nc.vector.max(out=rowc[:, t, mt], in_=cp[:, mt])
                t01 = work.tile([P, GS], BF16, tag="t01", name="t01")
                nc.vector.tensor_max(t01[:], cp[:, 0], cp[:, 1])
                nc.vector.tensor_max(maccb[:, noff], cp[:, 2], cp[:, 3])
                nc.vector.tensor_max(maccb[:, noff], maccb[:, noff], t01[:])
                nc.sync.dma_start(out=macc_ap[noff], in_=maccb[:, noff])
                noff += 1
            if t == 8:
                nc.scalar.dma_start(out=rowc_ap[:, 0:9], in_=rowc[:, 0:9])
            elif t == NBLK - 2:
                nc.scalar.dma_start(out=rowc_ap[:, 9 : NBLK - 1], in_=rowc[:, 9 : NBLK - 1])

        nc.scalar.dma_start(out=rowc_ap[:, NBLK - 1 : NBLK], in_=rowc[:, NBLK - 1 : NBLK])


def build_bass():
    nc = bacc.Bacc(
        "TRN2",
        target_bir_lowering=False,
        debug=False,
        enable_asserts=True,
        num_devices=NCORES,
    )
    x_t = nc.dram_tensor("xq", [KT, P, B], FP8, kind="ExternalInput").ap()
    rowc_t = nc.dram_tensor(
        "rowc", [P, NBLK, MT4, 8], F32, kind="ExternalOutput"
    ).ap()
    macc_t = nc.dram_tensor("macc", [NOFF, P, GS], BF16, kind="ExternalOutput").ap()
    with tile.TileContext(nc) as tc:
        emit_kernel(tc, x_t, rowc_t, macc_t)
    nc.compile()
    return nc


def make_in_maps(x: np.ndarray):
    norm = np.linalg.norm(x, axis=1, keepdims=True)
    xn = x / np.maximum(norm, 1e-8)
    q = (SCALE * xn).astype(ml_dtypes.float8_e4m3)
    # [KT, P, B]: element [k, p, r] = q[r, k*128 + p]  (transposed layout)
    xt = np.ascontiguousarray(q.reshape(B, KT, P).transpose(1, 2, 0))
    # core c sees the row axis rotated by c*GS: its strip s = global (s+c)%16
    return [
        {"xq": np.ascontiguousarray(np.roll(xt, -c * GS, axis=2))}
        for c in range(NCORES)
    ]


def reduce_outputs(results):
    cand = np.full((B, 16), -np.inf, np.float32)
    nsrc = np.zeros(B, np.int32)

    def put(rows, vals):
        cand[rows, nsrc[rows]] = vals
        nsrc[rows] += 1

    for c in range(NCORES):
        rowc = results[c]["rowc"].astype(np.float32)  # [P, NBLK, MT4, 8]
        macc = results[c]["macc"].astype(np.float32)  # [NOFF, P, GS]
        noff = 0
        for t, (a, b) in enumerate(TEMPLATE):
            ga, gb = (a + c) % NS, (b + c) % NS
            for mt in range(MT4):
                rows = np.arange(ga * GS + mt * P, ga * GS + (mt + 1) * P)
                if ga == gb:
                    # top-1 is the row's self-dot; top-2 is the candidate
                    put(rows, rowc[:, t, mt, 1])
                else:
                    put(rows, rowc[:, t, mt, 0])
            if ga != gb:
                rows = np.arange(gb * GS, (gb + 1) * GS)
                put(rows, macc[noff].max(axis=0))
                noff += 1

    assert (nsrc == 16).all()
    m2 = cand.max(axis=1).astype(np.float64)
    d2 = 2.0 - 2.0 * m2 / (SCALE * SCALE)
    loss = float(np.mean(-0.5 * np.log(d2)))
    return np.array(loss, dtype=np.float32)


_LAST_RESULTS = None  # BassKernelResults of the most recent run (for test.py)


def run(x: np.ndarray, trace: bool = False):
    global _LAST_RESULTS
    nc = build_bass()
    res = bass_utils.run_bass_kernel_spmd(
        nc,
        make_in_maps(x),
        core_ids=list(range(NCORES)),
        trace=trace,
        trace_cores=list(range(NCORES)) if trace else None,
    )
    _LAST_RESULTS = res
    return reduce_outputs(res.results)


def kernel(**inputs) -> np.ndarray:
    x = np.asarray(inputs["student_output"], dtype=np.float32)
    assert x.shape == (B, D), x.shape
    return run(x, trace=False)


if __name__ == "__main__":
    rng = np.random.default_rng(0)
    x = rng.standard_normal((B, D), dtype=np.float32)
    print(kernel(student_output=x))


# revision 24
# speedup vs baseline: 1.2382x; 1.0166x over previous
"""KoLeo loss kernel for Trainium2 (8 NeuronCores, Bass/Tile).

fp8 DoubleRow + symmetric-Gram edition.

reference semantics:
    x = student_output / max(||row||_2, 1e-8)        # [B, D] row-normalize
    dots = x @ x.T ; dots[i,i] = -1
    nn = argmax(dots, axis=1)
    d_i = || x_i - x_nn(i) + 1e-8 ||_2
    loss = mean(-log(d_i + 1e-8))

Strategy:
  * Host pre-normalizes rows in fp32, scales by S=128, quantizes to fp8
    e4m3 (TRN FP8_EXP4 max normal 240 > S) and ships the transposed
    layout [KT=8, 128, B].  End-to-end numpy-validated rel err 1.4e-4.
  * dots is symmetric: only the upper triangle of the 16x16 grid of
    [512 x 512] blocks is computed -- 136 blocks, 17 per core.  All
    cores run the IDENTICAL block template
        {(0,0), (8,8), (0,8)} + {(0,d), (8,8+d) : d=1..7}
    over a column-strip ROTATED copy of x (core c's strip s = global
    strip (s+c) mod 16).  The 8 rotations tile all 136 blocks exactly
    once (verified), so the NEFF is the same for every core and only
    the input data differs.
  * Each [512x512] block: 16 fp8 DoubleRow matmuls (2 k-tiles per MM,
    2x bf16 PE throughput) into 4 psum tiles [128,512].  ACT drains
    each psum tile to a bf16 SBUF copy; DVE max8 takes per-row tile
    maxima (row side); for off-diagonal blocks GPSIMD reduces the 4
    bf16 copies elementwise to macc[128,512] (column side), which is
    DMA'd to DRAM.
  * Host combine: for each global row, its NN dot is the max over the
    16 candidate values it receives (row-side tile maxima where the
    row's strip is the block's row side; partition-maxima of macc
    where it is the column side; for diagonal blocks the top-1 is the
    row's self-dot ~S^2 and the top-2 value is the candidate).  Then
    d^2 = 2 - 2 m~ / S^2, loss = mean(-0.5 log d^2).
"""

import numpy as np
import ml_dtypes

import concourse.bacc as bacc
import concourse.bass as bass
import concourse.mybir as mybir
import concourse.tile as tile
from concourse import bass_utils

B, D, P = 8192, 1024, 128
NCORES = 8
KT = D // P          # 8 contraction tiles
GS = 512             # strip size (block edge, also moving free dim)
NS = B // GS         # 16 strips
MT4 = GS // P        # 4 row chunks per block
SCALE = 128.0        # fp8 pre-scale; self-dot ~ S^2

# 17 blocks per core, ordered so the needed strips arrive incrementally
# and the LAST block is diagonal (cheapest drain tail: no mirror chain).
TEMPLATE = (
    [(0, 0)]
    + [(0, b) for b in range(1, 9)]
    + [(8, b) for b in range(9, 16)]
    + [(8, 8)]
)
NBLK = len(TEMPLATE)           # 17
OFF_SLOTS = [t for t, (a, b) in enumerate(TEMPLATE) if a != b]
NOFF = len(OFF_SLOTS)          # 15

F32 = mybir.dt.float32
BF16 = mybir.dt.bfloat16
FP8 = mybir.dt.float8e4
DR = mybir.MatmulPerfMode.DoubleRow


def emit_kernel(tc, x_ap, rowc_ap, macc_ap):
    nc = tc.nc
    with (
        tc.tile_pool(name="big", bufs=1) as big,
        tc.tile_pool(name="work", bufs=4) as work,
        tc.tile_pool(name="ps", bufs=2, space="PSUM") as pp,
    ):
        xT = big.tile([P, KT, B], FP8)
        # dedicated stationary-operand copy of strips 0 and 8 so LDWEIGHTS
        # reads never contend with the moving-operand reads of xT
        wT = big.tile([P, KT, 2, GS], FP8)
        rowc = big.tile([P, NBLK, MT4, 8], F32)
        # one dedicated macc slot per off-diagonal block: the outgoing DMAs
        # can lag arbitrarily without ever stalling the compute pipeline
        maccb = big.tile([P, NOFF, GS], BF16)
        warm = big.tile([P, GS], FP8)

        nc.vector.memset(warm[:], 1.0)

        # --- input DMA: one big multi-k DMA per 2-strip chunk (a single
        # InstDMACopy fans out across all 16 SDMA engines), all on the SP
        # queue so the ACT queue stays free for the psum-drain copies.
        # Order: weights strip 0, chunk(s0,s1), weights strip 8, chunks.
        nc.sync.dma_start(out=xT[:, :, 0:GS], in_=x_ap[:, :, 0:GS])
        nc.sync.dma_start(out=wT[:, :, 0], in_=x_ap[:, :, 0:GS])
        nc.sync.dma_start(out=xT[:, :, GS : 2 * GS], in_=x_ap[:, :, GS : 2 * GS])
        nc.sync.dma_start(out=wT[:, :, 1], in_=x_ap[:, :, 8 * GS : 9 * GS])
        for ch in range(1, NS // 2):
            cb = slice(ch * 2 * GS, (ch + 1) * 2 * GS)
            nc.sync.dma_start(out=xT[:, :, cb], in_=x_ap[:, :, cb])

        # --- PE/HAM pre-warm on the memset tile during the first DMAs.
        wps = pp.tile([P, GS], F32, tag="ps_m0", name="wps")
        for _ in range(14):
            nc.tensor.matmul(wps[:], warm[:, :P], warm[:], start=True, stop=True)

        # --- 17 symmetric blocks ------------------------------------------
        noff = 0
        for t, (a, b) in enumerate(TEMPLATE):
            ai = 0 if a == 0 else 1
            pss = [
                pp.tile([P, GS], F32, tag=f"ps_m{mt}", name=f"ps_m{mt}")
                for mt in range(MT4)
            ]
            for kk in range(KT // 2):
                ks = slice(2 * kk, 2 * kk + 2)
                for mt in range(MT4):
                    # block 0 runs during HAM warm-up where cadence doesn't
                    # matter; reading weights from xT there unblocks the
                    # first matmul from the wT DMA entirely.
                    w = (
                        xT[:, ks, mt * P : (mt + 1) * P]
                        if t == 0
                        else wT[:, ks, ai, mt * P : (mt + 1) * P]
                    )
                    nc.tensor.matmul(
                        pss[mt][:],
                        w,
                        xT[:, ks, b * GS : (b + 1) * GS],
                        start=(kk == 0),
                        stop=(kk == KT // 2 - 1),
                        perf_mode=DR,
                    )
            if a == b:
                # diagonal: row-side only, max8 straight from PSUM
                for mt in range(MT4):
                    nc.vector.max(out=rowc[:, t, mt], in_=pss[mt][:])
            else:
                cp = work.tile([P, MT4, GS], BF16, tag="cp", name="cp")
                for mt in range(MT4):
                    nc.scalar.copy(cp[:, mt], pss[mt][:])
                    nc.vector.max(out=rowc[:, t, mt], in_=cp[:, mt])
                t01 = work.tile([P, GS], BF16, tag="t01", name="t01")
                nc.vector.tensor_max(t01[:], cp[:, 0], cp[:, 1])
                nc.vector.tensor_max(maccb[:, noff], cp[:, 2], cp[:, 3])
                nc.vector.tensor_max(maccb[:, noff], maccb[:, noff], t01[:])
                nc.sync.dma_start(out=macc_ap[noff], in_=maccb[:, noff])
                noff += 1
            if t == 8:
                nc.scalar.dma_start(out=rowc_ap[:, 0:9], in_=rowc[:, 0:9])
            elif t == NBLK - 2:
                nc.scalar.dma_start(out=rowc_ap[:, 9 : NBLK - 1], in_=rowc[:, 9 : NBLK - 1])

        nc.scalar.dma_start(out=rowc_ap[:, NBLK - 1 : NBLK], in_=rowc[:, NBLK - 1 : NBLK])


def build_bass():
    nc = bacc.Bacc(
        "TRN2",
        target_bir_lowering=False,
        debug=False,
        enable_asserts=True,
        num_devices=NCORES,
    )
    x_t = nc.dram_tensor("xq", [KT, P, B], FP8, kind="ExternalInput").ap()
    rowc_t = nc.dram_tensor(
        "rowc", [P, NBLK, MT4, 8], F32, kind="ExternalOutput"
    ).ap()
    macc_t = nc.dram_tensor("macc", [NOFF, P, GS], BF16, kind="ExternalOutput").ap()
    with tile.TileContext(nc) as tc:
        emit_kernel(tc, x_t, rowc_t, macc_t)
    nc.compile()
    return nc


def make_in_maps(x: np.ndarray):
    norm = np.linalg.norm(x, axis=1, keepdims=True)
    xn = x / np.maximum(norm, 1e-8)
    q = (SCALE * xn).astype(ml_dtypes.float8_e4m3)
    # [KT, P, B]: element [k, p, r] = q[r, k*128 + p]  (transposed layout)
    xt = np.ascontiguousarray(q.reshape(B, KT, P).transpose(1, 2, 0))
    # core c sees the row axis rotated by c*GS: its strip s = global (s+c)%16
    return [
        {"xq": np.ascontiguousarray(np.roll(xt, -c * GS, axis=2))}
        for c in range(NCORES)
    ]


def reduce_outputs(results):
    cand = np.full((B, 16), -np.inf, np.float32)
    nsrc = np.zeros(B, np.int32)

    def put(rows, vals):
        cand[rows, nsrc[rows]] = vals
        nsrc[rows] += 1

    for c in range(NCORES):
        rowc = results[c]["rowc"].astype(np.float32)  # [P, NBLK, MT4, 8]
        macc = results[c]["macc"].astype(np.float32)  # [NOFF, P, GS]
        noff = 0
        for t, (a, b) in enumerate(TEMPLATE):
            ga, gb = (a + c) % NS, (b + c) % NS
            for mt in range(MT4):
                rows = np.arange(ga * GS + mt * P, ga * GS + (mt + 1) * P)
                if ga == gb:
                    # top-1 is the row's self-dot; top-2 is the candidate
                    put(rows, rowc[:, t, mt, 1])
                else:
                    put(rows, rowc[:, t, mt, 0])
            if ga != gb:
                rows = np.arange(gb * GS, (gb + 1) * GS)
                put(rows, macc[noff].max(axis=0))
                noff += 1

    assert (nsrc == 16).all()
    m2 = cand.max(axis=1).astype(np.float64)
    d2 = 2.0 - 2.0 * m2 / (SCALE * SCALE)
    loss = float(np.mean(-0.5 * np.log(d2)))
    return np.array(loss, dtype=np.float32)


_LAST_RESULTS = None  # BassKernelResults of the most recent run (for test.py)


def run(x: np.ndarray, trace: bool = False):
    global _LAST_RESULTS
    nc = build_bass()
    res = bass_utils.run_bass_kernel_spmd(
        nc,
        make_in_maps(x),
        core_ids=list(range(NCORES)),
        trace=trace,
        trace_cores=list(range(NCORES)) if trace else None,
    )
    _LAST_RESULTS = res
    return reduce_outputs(res.results)


def kernel(**inputs) -> np.ndarray:
    x = np.asarray(inputs["student_output"], dtype=np.float32)
    assert x.shape == (B, D), x.shape
    return run(x, trace=False)


if __name__ == "__main__":
    rng = np.random.default_rng(0)
    x = rng.standard_normal((B, D), dtype=np.float32)
    print(kernel(student_output=x))
